# revision 1
# baseline (speedup 1.0000x reference)
"""Trainium2 Bass kernel for nn_EdgeDecoder_lgcn (gnn_message_passing).

Computation (reference):
    logit = tanh(z_src @ W1 + b1) @ w2            # [NS]
    beta  = softmax(where(mask, logit, -inf), 1)  # [G, NS]
    agg   = beta @ z_src                          # [G, H]
    scores= agg @ z_dst.T                         # [G, ND]

Sharding: NS is split across the 8 cores for phase 1 (each core computes
logits for its slice and the partial masked-exp sums U_part = w.T@[z|1]
with w[i,g] = mask[g,i]*exp(logit[i])), a 66 KB AllReduce combines
(U, s), and ND is split across the cores for phase 2
(scores_slice = (U/s) @ z_dst_slice.T).

Host-side prep: the mask slice ships as bf16 (0/1 exact, half the DMA,
PE-transposable), and the z_dst slice ships pre-transposed [H, NDL] so
phase 2 needs no on-device transposes.

No max-subtraction is needed in the softmax: logit ~ N(0, 0.62), so
exp(logit) is far from overflow and fp32 exp/sums match the reference
to ~1e-6.
"""

import numpy as np

NS = 50000
ND = 50000
G = 128
H = 128
NCORES = 8
TPD = 49                 # 128-row i-tiles per device
NSL = TPD * 128          # 6272 rows per device slice
NSP = NCORES * NSL       # 50176 padded NS
NDL = NSL
NDP = NSP
GRP = 4                  # i-tiles batched per 512-wide group
NGRP = (TPD + GRP - 1) // GRP

# dtype knobs (fp32 = exact, fp32r = fast reduced-precision matmul)
T_MM_F32R = True        # t = z @ W1          (N=512 moving)
SC_MM_F32R = True       # scores = U @ zdT    (N=512 moving)

_CACHE = {}


def _build_module(no_collective=False, num_devices=NCORES):
    import concourse.bacc as bacc
    import concourse.mybir as mybir
    import concourse.tile as tile
    from concourse import masks

    fp32 = mybir.dt.float32
    fp32r = mybir.dt.float32r
    bf16 = mybir.dt.bfloat16

    t_dt = fp32r if T_MM_F32R else fp32
    sc_dt = fp32r if SC_MM_F32R else fp32

    nc = bacc.Bacc(
        "TRN2", target_bir_lowering=False, debug=False, num_devices=num_devices
    )

    zs = nc.dram_tensor("zs", [NSL, H], fp32, kind="ExternalInput").ap()
    sym = nc.dram_tensor("sym", [G, NSL], bf16, kind="ExternalInput").ap()
    zdt = nc.dram_tensor("zdt", [H, NDL], sc_dt, kind="ExternalInput").ap()
    W1 = nc.dram_tensor("W1", [H, H], fp32, kind="ExternalInput").ap()
    b1 = nc.dram_tensor("b1", [H, 1], fp32, kind="ExternalInput").ap()
    w2 = nc.dram_tensor("w2", [H, 1], fp32, kind="ExternalInput").ap()
    out = nc.dram_tensor("scores", [G, NDL], fp32, kind="ExternalOutput").ap()

    cc_in = nc.dram_tensor("cc_in", [G, H + 1], fp32)
    cc_out = nc.dram_tensor("cc_out", [G, H + 1], fp32, addr_space="Shared")

    Tanh = mybir.ActivationFunctionType.Tanh
    Exp = mybir.ActivationFunctionType.Exp

    with tile.TileContext(nc) as tc:
        with (
            tc.tile_pool(name="const", bufs=1) as cpool,
            tc.tile_pool(name="big", bufs=1) as big,
            tc.tile_pool(name="sbA", bufs=4) as sbA,
            tc.tile_pool(name="sbB", bufs=4) as sbB,
            tc.tile_pool(name="sbC", bufs=1) as sbC,
            tc.tile_pool(name="sbD", bufs=4) as sbD,
        ):
            # ---- constants ----
            ident = cpool.tile([128, 128], fp32)
            masks.make_identity(nc, ident[:])
            ident_bf = cpool.tile([128, 128], bf16)
            masks.make_identity(nc, ident_bf[:])
            W1_sb = cpool.tile([H, H], fp32)          # [h, h'] natural
            nc.sync.dma_start(out=W1_sb[:], in_=W1)
            W1t_sb = cpool.tile([H, H], t_dt)
            nc.scalar.copy(W1t_sb[:], W1_sb[:])
            b1_sb = cpool.tile([H, 1], fp32)
            nc.sync.dma_start(out=b1_sb[:], in_=b1)
            w2_sb = cpool.tile([H, 1], fp32)
            nc.sync.dma_start(out=w2_sb[:], in_=w2)
            ones_sb = cpool.tile([H, 1], fp32)
            nc.vector.memset(ones_sb[:], 1.0)

            # ---- bulk inputs (chunked so compute can start early) ----
            # Zs1: partition p holds rows i = 49p + c, c in [0,49), each row
            # followed by a literal 1.0 -> tile c is [:, 129c : 129c+129]
            # = [z_i | 1], giving U and s from one matmul.
            Zs1_sb = big.tile([128, TPD * 129], fp32)
            Zs1v = Zs1_sb[:].rearrange("p (n x) -> p n x", x=129)
            zsv = zs.rearrange("(p n) h -> p n h", p=128)
            Ms_sb = big.tile([128, NSL], bf16)
            # mask col i = 49j + c  ->  [g, j, c] view, c innermost
            Msv = Ms_sb[:].rearrange("g (j c) -> g j c", c=TPD)
            ZdT_sb = big.tile([128, NDL], sc_dt)

            bounds = [0, 4, 10, 17, 25, 33, 41, TPD]
            for k in range(len(bounds) - 1):
                lo, hi = bounds[k], bounds[k + 1]
                nc.sync.dma_start(
                    out=Zs1v[:, lo:hi, 0:128], in_=zsv[:, lo:hi, :]
                )
                nc.any.memset(Zs1v[:, lo:hi, 128:129], 1.0)
            for lo, hi in [(0, 17), (17, 33), (33, TPD)]:
                nc.sync.dma_start(
                    out=Ms_sb[:, lo * 128 : hi * 128],
                    in_=sym[:, lo * 128 : hi * 128],
                )

            e_sb = cpool.tile([128, TPD], fp32)

            # ---- pass A (logits) interleaved with pass B (U/s accum) ----
            ab_pools = tc.tile_pool(name="zt_ps", bufs=2, space="PSUM")
            ztp = ab_pools.__enter__()
            ttp_cm = tc.tile_pool(name="t_ps", bufs=2, space="PSUM")
            ttp = ttp_cm.__enter__()
            mtp_cm = tc.tile_pool(name="mt_ps", bufs=3, space="PSUM")
            mtp = mtp_cm.__enter__()
            upl_cm = tc.tile_pool(name="u_ps", bufs=1, space="PSUM")
            upl = upl_cm.__enter__()
            U_ps = upl.tile([G, H + 1], fp32)
            for g in range(NGRP):
                tiles = list(range(g * GRP, min((g + 1) * GRP, TPD)))
                n_t = len(tiles)
                W = n_t * 128
                c0 = tiles[0]
                zT_ps = ztp.tile([128, GRP * 128], fp32, tag="zt")
                for j, c in enumerate(tiles):
                    nc.tensor.transpose(
                        zT_ps[:, j * 128 : (j + 1) * 128],
                        Zs1_sb[:, c * 129 : c * 129 + 128],
                        ident[:],
                    )
                zT_sb = sbA.tile([128, GRP * 128], t_dt, tag="zts")
                nc.any.tensor_copy(zT_sb[:, :W], zT_ps[:, :W])
                t_ps = ttp.tile([128, GRP * 128], fp32, tag="tps")
                nc.tensor.matmul(
                    t_ps[:, :W], W1t_sb[:], zT_sb[:, :W], start=True, stop=True
                )
                tanh_sb = sbA.tile([128, GRP * 128], fp32, tag="tanh")
                nc.scalar.activation(
                    tanh_sb[:, :W], t_ps[:, :W], Tanh, bias=b1_sb[:], scale=1.0
                )
                q_sb = sbA.tile([128, GRP * 128], fp32, tag="q")
                nc.vector.tensor_scalar_mul(q_sb[:, :W], tanh_sb[:, :W], w2_sb[:])
                if g in (5, 8):
                    half = NDL // 2
                    s0 = 0 if g == 5 else half
                    nc.sync.dma_start(
                        out=ZdT_sb[:, s0 : s0 + half],
                        in_=zdt[:, s0 : s0 + half],
                    )
                lg_ps = mtp.tile([128, GRP], fp32, tag="mt")
                for j, c in enumerate(tiles):
                    nc.tensor.matmul(
                        lg_ps[:, j : j + 1],
                        q_sb[:, j * 128 : (j + 1) * 128],
                        ones_sb[:],
                        start=True,
                        stop=True,
                    )
                nc.scalar.activation(e_sb[:, c0 : c0 + n_t], lg_ps[:, :n_t], Exp)

                # pass B for this group's tiles: maskT, w = maskT*e, U +=
                mT_ps = mtp.tile([128, GRP * 128], bf16, tag="mt")
                for j, c in enumerate(tiles):
                    nc.tensor.transpose(
                        mT_ps[:, j * 128 : (j + 1) * 128],
                        Msv[:, :, c],
                        ident_bf[:],
                    )
                w_sb = sbB.tile([128, GRP * 128], fp32, tag="w")
                nc.vector.tensor_mul(
                    w_sb[:, :W].rearrange("p (c i) -> p c i", i=128),
                    mT_ps[:, :W].rearrange("p (c i) -> p c i", i=128),
                    e_sb[:, c0 : c0 + n_t].unsqueeze(2).to_broadcast(
                        [128, n_t, 128]
                    ),
                )
                for j, c in enumerate(tiles):
                    nc.tensor.matmul(
                        U_ps[:],
                        w_sb[:, j * 128 : (j + 1) * 128],
                        Zs1_sb[:, c * 129 : (c + 1) * 129],
                        start=(c == 0),
                        stop=(c == TPD - 1),
                    )

            # ---- pass C: AllReduce (U, s) and prep (U^T, 1/s) ----
            Us_sb = sbC.tile([G, H + 1], fp32)
            nc.any.tensor_copy(Us_sb[:], U_ps[:])
            nc.sync.dma_start(out=cc_in.ap(), in_=Us_sb[:])
            if no_collective:
                nc.sync.dma_start(out=cc_out.ap(), in_=cc_in.ap())
            else:
                nc.gpsimd.collective_compute(
                    "AllReduce",
                    mybir.AluOpType.add,
                    replica_groups=[list(range(NCORES))],
                    ins=[cc_in.ap().opt()],
                    outs=[cc_out.ap().opt()],
                )
            Usum_sb = sbC.tile([G, H + 1], fp32)
            nc.sync.dma_start(out=Usum_sb[:], in_=cc_out.ap())
            rs_sb = sbC.tile([G, 1], fp32)
            nc.vector.reciprocal(rs_sb[:], Usum_sb[:, H : H + 1])
            UT_ps = ztp.tile([128, GRP * 128], fp32, tag="zt")
            nc.tensor.transpose(UT_ps[:, 0:128], Usum_sb[:, :H], ident[:])
            UT_sb = sbC.tile([H, G], sc_dt)
            nc.scalar.copy(UT_sb[:], UT_ps[:, 0:128])
            upl_cm.__exit__(None, None, None)
            mtp_cm.__exit__(None, None, None)
            ttp_cm.__exit__(None, None, None)
            ab_pools.__exit__(None, None, None)
            dps_cm = tc.tile_pool(name="d_ps", bufs=4, space="PSUM")
            dps = dps_cm.__enter__()

            # ---- pass D: scores slice (z_dst arrives pre-transposed) ----
            for m in range(NGRP):
                lo = m * GRP * 128
                W = min(GRP * 128, NDL - lo)
                sc_ps = dps.tile([G, GRP * 128], fp32, tag="sc")
                nc.tensor.matmul(
                    sc_ps[:, :W],
                    UT_sb[:],
                    ZdT_sb[:, lo : lo + W],
                    start=True,
                    stop=True,
                )
                o_sb = sbD.tile([G, GRP * 128], fp32, tag="o")
                nc.any.tensor_scalar_mul(o_sb[:, :W], sc_ps[:, :W], rs_sb[:])
                eng = nc.sync if m % 2 == 0 else nc.scalar
                eng.dma_start(out=out[:, lo : lo + W], in_=o_sb[:, :W])
            dps_cm.__exit__(None, None, None)

    nc.compile()
    return nc


def _get_module():
    if "nc" not in _CACHE:
        _CACHE["nc"] = _build_module()
    return _CACHE["nc"]


def make_in_maps(z_src, z_dst, sym_indexs, W1, b1, w2):
    import ml_dtypes

    z_src = np.ascontiguousarray(np.asarray(z_src, dtype=np.float32))
    z_dst = np.ascontiguousarray(np.asarray(z_dst, dtype=np.float32))
    sym_indexs = np.asarray(sym_indexs)
    W1 = np.ascontiguousarray(np.asarray(W1, dtype=np.float32))
    b1 = np.ascontiguousarray(np.asarray(b1, dtype=np.float32)).reshape(H, 1)
    w2 = np.ascontiguousarray(np.asarray(w2, dtype=np.float32)).reshape(H, 1)

    zsp = np.zeros((NSP, H), dtype=np.float32)
    zsp[:NS] = z_src
    symp = np.zeros((G, NSP), dtype=ml_dtypes.bfloat16)
    symp[:, :NS] = sym_indexs.astype(ml_dtypes.bfloat16)
    zdtp = np.zeros((H, NDP), dtype=np.float32)
    zdtp[:, :ND] = z_dst.T

    in_maps = []
    for k in range(NCORES):
        lo = k * NSL
        in_maps.append(
            {
                "zs": np.ascontiguousarray(zsp[lo : lo + NSL]),
                "sym": np.ascontiguousarray(symp[:, lo : lo + NSL]),
                "zdt": np.ascontiguousarray(zdtp[:, lo : lo + NDL]),
                "W1": W1,
                "b1": b1,
                "w2": w2,
            }
        )
    return in_maps


def kernel(z_src, z_dst, sym_indexs, W1, b1, w2):
    from concourse import bass_utils

    in_maps = make_in_maps(z_src, z_dst, sym_indexs, W1, b1, w2)
    nc = _get_module()
    res = bass_utils.run_bass_kernel_spmd(
        nc, in_maps, core_ids=list(range(NCORES))
    )
    scores = np.empty((G, NDP), dtype=np.float32)
    for k in range(NCORES):
        scores[:, k * NDL : (k + 1) * NDL] = res.results[k]["scores"]
    return scores[:, :ND]


if __name__ == "__main__":
    rng = np.random.default_rng(0)
    inputs = {
        "z_src": rng.standard_normal((NS, H), dtype=np.float32),
        "z_dst": rng.standard_normal((ND, H), dtype=np.float32),
        "sym_indexs": rng.integers(0, 2, (G, NS), dtype=np.int32),
        "W1": rng.standard_normal((H, H), dtype=np.float32) / np.sqrt(H),
        "b1": np.zeros(H, dtype=np.float32),
        "w2": rng.standard_normal(H, dtype=np.float32) / np.sqrt(H),
    }
    out = kernel(**inputs)
    print(out.shape, out.dtype, np.abs(out).max())

